# revision 4
# baseline (speedup 1.0000x reference)
"""CRD-style contrastive loss + EMA memory-bank update on 8 TRN2 NeuronCores.

Strategy (data-parallel over batch, bank replicated per core):
  - Each core owns B/8 = 64 batch rows and gathers their 64*4096 negative rows
    from its replica of the 1.2M x 128 memory bank via dma_gather
    (int16 windowed indices, 1024 rows per instruction, 4 SWDGE queues).
  - Slot layout: within a window, each SBUF partition p is assigned to one
    batch row b(p, w) (slots apportioned to each b's sample count), so the
    s-multiplier is a per-partition constant tile broadcast over columns.
  - Row norms and dots on device: ACT square, DVE mult + segmented reduces,
    batched sqrt/recip/exp tail; per-(partition, window) exp-sums via DVE.
  - Host finishes: per-b sums of the (p, w) partials, logsumexp, and the
    scatter of the 512 re-normalized EMA rows into a copy of the bank.
"""
import numpy as np
from contextlib import ExitStack

import concourse.bass as bass
import concourse.bacc as bacc
import concourse.tile as tile
from concourse import mybir
from concourse.bass_utils import run_bass_kernel_spmd

N_DATA = 1_200_000
FEAT = 128
BATCH = 512
N_NEG = 4096
TEMP = 0.07
MOMENTUM = 0.5

NCORES = 8
BPC = BATCH // NCORES  # 64 batch rows per core
WIN = 32768            # dma_gather int16 index window (rows)
NWIN = (N_DATA + WIN - 1) // WIN  # 37
NI = 1024              # indices per dma_gather instruction
CPI = NI // 128        # 8 columns (of 128 slots) per instruction

_F32 = mybir.dt.float32
_I16 = mybir.dt.int16
_I32 = mybir.dt.int32

_prog_cache = {}
_last_in_maps = None


def _apportion(cnts):
    """Largest-remainder apportionment of 128 partition slots to the active
    batch rows, proportional to their sample counts. Returns int array of
    slots per active b (each >= 1, sums to 128)."""
    nb = len(cnts)
    total = cnts.sum()
    raw = 128.0 * cnts / total
    base = np.maximum(np.floor(raw).astype(np.int64), 1)
    while base.sum() > 128:  # rare: the >=1 floor overshot
        j = np.argmax(base)
        base[j] -= 1
    rem = 128 - base.sum()
    if rem > 0:
        frac = raw - np.floor(raw)
        order = np.argsort(-frac, kind="stable")
        for j in order[:rem]:
            base[j] += 1
    return base


def _route_core(neg_c):
    """neg_c: [BPC, N_NEG] int64 global row indices.

    Returns per-window dicts with the slot layout:
      locidx  [128, ncols_w] int16 local indices
      mask    [128, ncols_w] float32 (1 = real sample)
      b_of_p  [128] int64 batch row owning each partition
    """
    w = neg_c // WIN
    loc = neg_c % WIN
    out = []
    for wv in range(NWIN):
        sel = w == wv
        cnt_b = sel.sum(axis=1)  # [BPC]
        active = np.nonzero(cnt_b)[0]
        if len(active) == 0:
            out.append(None)
            continue
        cnts = cnt_b[active]
        slots = _apportion(cnts)
        ncols_w = int(np.max((cnts + slots - 1) // slots))
        b_of_p = np.zeros(128, dtype=np.int64)
        locidx = np.zeros((128, ncols_w), dtype=np.int16)
        mask = np.zeros((128, ncols_w), dtype=np.float32)
        p0 = 0
        cols_ar = np.arange(ncols_w)
        for bj, b in enumerate(active):
            ns = int(slots[bj])
            cb = int(cnts[bj])
            q = loc[b][sel[b]].astype(np.int16)  # [cb] local indices
            # slot row r of this b at column j consumes sample j*ns + r
            pos = cols_ar[None, :] * ns + np.arange(ns)[:, None]  # [ns, ncols_w]
            valid = pos < cb
            locidx[p0 : p0 + ns] = q[np.minimum(pos, cb - 1)]
            mask[p0 : p0 + ns] = valid
            b_of_p[p0 : p0 + ns] = b
            p0 += ns
        assert p0 == 128
        out.append({"locidx": locidx, "mask": mask, "b_of_p": b_of_p})
    return out


def _build_program(nipw, ncols):
    nc = bacc.Bacc(
        "TRN2",
        target_bir_lowering=False,
        debug=False,
        num_swdge_queues=4,
    )
    ninstr = sum(nipw)

    bank = nc.dram_tensor("bank", [N_DATA, FEAT], _F32, kind="ExternalInput").ap()
    idx16 = nc.dram_tensor("idx16", [128, ninstr, NI // 16], _I16, kind="ExternalInput").ap()
    s2d = nc.dram_tensor("s2d", [128, NWIN, FEAT], _F32, kind="ExternalInput").ap()
    maskd = nc.dram_tensor("maskd", [128, ncols], _F32, kind="ExternalInput").ap()
    s_raw = nc.dram_tensor("s_raw", [BPC, FEAT], _F32, kind="ExternalInput").ap()
    t_raw = nc.dram_tensor("t_raw", [BPC, FEAT], _F32, kind="ExternalInput").ap()
    emaidx = nc.dram_tensor("emaidx", [BPC, 1], _I32, kind="ExternalInput").ap()

    pw_o = nc.dram_tensor("pw", [128, NWIN], _F32, kind="ExternalOutput").ap()
    pos_o = nc.dram_tensor("pos", [BPC, 1], _F32, kind="ExternalOutput").ap()
    upd_o = nc.dram_tensor("upd", [BPC, FEAT], _F32, kind="ExternalOutput").ap()

    SQ = mybir.ActivationFunctionType.Square
    SQRT = mybir.ActivationFunctionType.Sqrt
    EXP = mybir.ActivationFunctionType.Exp
    CPY = mybir.ActivationFunctionType.Copy
    ADD = mybir.AluOpType.add
    MULT = mybir.AluOpType.mult
    AXX = mybir.AxisListType.X

    with tile.TileContext(nc) as tc, ExitStack() as ctx:
        persist = ctx.enter_context(tc.tile_pool(name="persist", bufs=1))
        gpool = ctx.enter_context(tc.tile_pool(name="gp", bufs=6))
        mpool = ctx.enter_context(tc.tile_pool(name="mp", bufs=3))
        qpool = ctx.enter_context(tc.tile_pool(name="qp", bufs=3))
        epool = ctx.enter_context(tc.tile_pool(name="ep", bufs=2))

        idx_t = persist.tile([128, ninstr, NI // 16], _I16)
        nc.sync.dma_start(out=idx_t[:], in_=idx16[:, :, :])
        s2_t = persist.tile([128, NWIN, FEAT], _F32)
        nc.sync.dma_start(out=s2_t[:], in_=s2d[:, :, :])
        mask_t = persist.tile([128, ncols], _F32)
        nc.sync.dma_start(out=mask_t[:], in_=maskd[:, :])

        Z = persist.tile([128, ncols], _F32)
        N2 = persist.tile([128, ncols], _F32)
        NR = persist.tile([128, ncols], _F32)
        PW = persist.tile([128, NWIN], _F32)

        i = 0
        col0 = []
        for w, cnt in enumerate(nipw):
            col0.append(i * CPI)
            if cnt == 0:
                continue
            lo = w * WIN
            hi = min(N_DATA, lo + WIN)
            win_ap = bank[lo:hi, :]
            for _ in range(cnt):
                g = gpool.tile([128, CPI, FEAT], _F32, tag="g")
                nc.gpsimd.dma_gather(
                    out_ap=g[:],
                    in_ap=win_ap,
                    idxs_ap=idx_t[:, i, :],
                    num_idxs=NI,
                    num_idxs_reg=NI,
                    elem_size=FEAT,
                    queue_num=i % 4,
                )
                gsq = qpool.tile([128, CPI, FEAT], _F32, tag="gsq")
                nc.scalar.activation(out=gsq[:], in_=g[:], func=SQ)
                m = mpool.tile([128, CPI, FEAT], _F32, tag="m")
                nc.vector.tensor_tensor(
                    out=m[:],
                    in0=g[:],
                    in1=s2_t[:, w : w + 1, :].to_broadcast([128, CPI, FEAT]),
                    op=MULT,
                )
                nc.vector.tensor_reduce(
                    out=Z[:, i * CPI : (i + 1) * CPI], in_=m[:], axis=AXX, op=ADD
                )
                nc.vector.tensor_reduce(
                    out=N2[:, i * CPI : (i + 1) * CPI], in_=gsq[:], axis=AXX, op=ADD
                )
                i += 1
        col0.append(i * CPI)

        # tail: logit = z * rsqrt(n2)  (1/TEMP folded into s2), exp, mask
        nc.scalar.activation(out=NR[:], in_=N2[:], func=SQRT)
        nc.vector.reciprocal(out=N2[:], in_=NR[:])
        nc.vector.tensor_tensor(out=NR[:], in0=Z[:], in1=N2[:], op=MULT)
        nc.scalar.activation(out=Z[:], in_=NR[:], func=EXP)
        nc.vector.tensor_tensor(out=NR[:], in0=Z[:], in1=mask_t[:], op=MULT)

        nc.vector.memset(PW[:], 0.0)
        for w in range(NWIN):
            a, bnd = col0[w], col0[w + 1]
            if bnd > a:
                nc.vector.tensor_reduce(
                    out=PW[:, w : w + 1], in_=NR[:, a:bnd], axis=AXX, op=ADD
                )
        nc.sync.dma_start(out=pw_o[:, :], in_=PW[:])

        # --- EMA update + positive logits (tiny) ---
        def l2norm_rows(x, pool, tagp):
            sq = pool.tile([BPC, FEAT], _F32, tag=tagp + "sq")
            nc.scalar.activation(out=sq[:], in_=x[:], func=SQ)
            n2 = pool.tile([BPC, 1], _F32, tag=tagp + "n2")
            nc.vector.tensor_reduce(out=n2[:], in_=sq[:], axis=AXX, op=ADD)
            nn = pool.tile([BPC, 1], _F32, tag=tagp + "n")
            nc.scalar.activation(out=nn[:], in_=n2[:], func=SQRT)
            rn = pool.tile([BPC, 1], _F32, tag=tagp + "rn")
            nc.vector.reciprocal(out=rn[:], in_=nn[:])
            o = pool.tile([BPC, FEAT], _F32, tag=tagp + "o")
            nc.scalar.activation(out=o[:], in_=x[:], func=CPY, scale=rn[:])
            return o

        s_t = epool.tile([BPC, FEAT], _F32, tag="s")
        nc.sync.dma_start(out=s_t[:], in_=s_raw[:, :])
        t_t = epool.tile([BPC, FEAT], _F32, tag="t")
        nc.sync.dma_start(out=t_t[:], in_=t_raw[:, :])
        ei_t = epool.tile([BPC, 1], _I32, tag="ei")
        nc.sync.dma_start(out=ei_t[:], in_=emaidx[:, :])
        mg = epool.tile([BPC, FEAT], _F32, tag="mg")
        nc.gpsimd.indirect_dma_start(
            out=mg[:],
            out_offset=None,
            in_=bank[:],
            in_offset=bass.IndirectOffsetOnAxis(ap=ei_t[:, :1], axis=0),
        )
        s_n = l2norm_rows(s_t, epool, "sn")
        t_n = l2norm_rows(t_t, epool, "tn")
        pm = epool.tile([BPC, FEAT], _F32, tag="pm")
        nc.vector.tensor_tensor(out=pm[:], in0=s_n[:], in1=t_n[:], op=MULT)
        pos_t = epool.tile([BPC, 1], _F32, tag="pos")
        nc.vector.tensor_reduce(out=pos_t[:], in_=pm[:], axis=AXX, op=ADD)
        nc.sync.dma_start(out=pos_o[:, :], in_=pos_t[:])
        mh = epool.tile([BPC, FEAT], _F32, tag="mh")
        nc.scalar.activation(out=mh[:], in_=mg[:], func=CPY, scale=MOMENTUM)
        th = epool.tile([BPC, FEAT], _F32, tag="th")
        nc.scalar.activation(out=th[:], in_=t_n[:], func=CPY, scale=1.0 - MOMENTUM)
        ub = epool.tile([BPC, FEAT], _F32, tag="ub")
        nc.vector.tensor_tensor(out=ub[:], in0=mh[:], in1=th[:], op=ADD)
        upd_t = l2norm_rows(ub, epool, "up")
        nc.sync.dma_start(out=upd_o[:, :], in_=upd_t[:])

    nc.compile()
    return nc


def kernel(student_feat, teacher_feat, memory_bank, indices, r):
    student = np.ascontiguousarray(np.asarray(student_feat, dtype=np.float32))
    teacher = np.ascontiguousarray(np.asarray(teacher_feat, dtype=np.float32))
    bank = np.ascontiguousarray(np.asarray(memory_bank, dtype=np.float32))
    idx = np.asarray(indices).astype(np.int64).reshape(BATCH)
    rr = np.asarray(r).astype(np.int64).reshape(BATCH, N_NEG)

    neg = rr + (rr >= idx[:, None])

    # host-side s (normalized, scaled by 1/TEMP) for the per-partition table
    sn = student / np.maximum(
        np.sqrt((student * student).sum(axis=1, keepdims=True)), 1e-12
    )
    sc_all = (sn / TEMP).astype(np.float32)

    routes = [_route_core(neg[c * BPC : (c + 1) * BPC]) for c in range(NCORES)]
    nipw = []
    for w in range(NWIN):
        mx = 0
        for c in range(NCORES):
            rw = routes[c][w]
            if rw is not None:
                mx = max(mx, rw["locidx"].shape[1])
        nipw.append((mx + CPI - 1) // CPI)
    ninstr = sum(nipw)
    ncols = ninstr * CPI

    key = (tuple(nipw), ncols)
    if key not in _prog_cache:
        _prog_cache[key] = _build_program(nipw, ncols)
    nc = _prog_cache[key]

    in_maps = []
    bmaps = []
    for c in range(NCORES):
        locidx = np.zeros((128, ncols), dtype=np.int16)
        maskd = np.zeros((128, ncols), dtype=np.float32)
        s2d = np.zeros((128, NWIN, FEAT), dtype=np.float32)
        b_of_pw = np.zeros((128, NWIN), dtype=np.int64)
        cp = 0
        for w in range(NWIN):
            want = nipw[w] * CPI
            rw = routes[c][w]
            if rw is not None:
                k = rw["locidx"].shape[1]
                locidx[:, cp : cp + k] = rw["locidx"]
                maskd[:, cp : cp + k] = rw["mask"]
                s2d[:, w, :] = sc_all[c * BPC + rw["b_of_p"]]
                b_of_pw[:, w] = rw["b_of_p"]
            cp += want
        # per-instruction int16 streams: t = c*128 + p -> (t%16, t//16), x8
        st = locidx.T.reshape(ninstr, NI)  # stream per instruction
        w16 = st.reshape(ninstr, NI // 16, 16).transpose(2, 0, 1)  # [16, ni, 64]
        idx16 = np.ascontiguousarray(np.tile(w16, (8, 1, 1)))

        in_maps.append(
            {
                "bank": bank,
                "idx16": idx16,
                "s2d": s2d,
                "maskd": maskd,
                "s_raw": student[c * BPC : (c + 1) * BPC],
                "t_raw": teacher[c * BPC : (c + 1) * BPC],
                "emaidx": idx[c * BPC : (c + 1) * BPC, None].astype(np.int32),
            }
        )
        bmaps.append(b_of_pw)

    global _last_in_maps
    _last_in_maps = in_maps
    res = run_bass_kernel_spmd(nc, in_maps, list(range(NCORES)))

    # --- host unshard / finish ---
    S_neg = np.zeros(BATCH, dtype=np.float64)
    pos = np.zeros(BATCH, dtype=np.float32)
    upd = np.zeros((BATCH, FEAT), dtype=np.float32)
    for c in range(NCORES):
        out = res.results[c]
        pw = np.asarray(out["pw"], dtype=np.float64).reshape(128, NWIN)
        S_neg[c * BPC : (c + 1) * BPC] = np.bincount(
            bmaps[c].ravel(), weights=pw.ravel(), minlength=BPC
        )
        pos[c * BPC : (c + 1) * BPC] = np.asarray(out["pos"], np.float32).reshape(BPC)
        upd[c * BPC : (c + 1) * BPC] = np.asarray(out["upd"], np.float32).reshape(
            BPC, FEAT
        )

    pos_l = pos.astype(np.float64) / TEMP
    total = S_neg + np.exp(pos_l)
    loss = np.float32(np.mean(np.log(total) - pos_l))

    new_bank = bank.copy()
    new_bank[idx] = upd
    return loss, new_bank
